# revision 1
# baseline (speedup 1.0000x reference)
"""CrossLayerTranscoder with global batch-wise top-k masking on 8 TRN2 cores.

Reference computation:
    pre = relu(x @ W_enc + b_enc)            [4096, 16384]
    keep the global top-(top_k * 4096) entries, zero the rest.

Device algorithm (single pass, dict-sharded over 8 cores):
  * GEMM in split-f32r precision: x and W are split into hi (11 mantissa
    bits) + lo parts; hi*hi + hi*lo + lo*hi at float32r full rate gives
    fp32-level accuracy at 3 cycles/row instead of fp32's 4.
  * Transposed orientation (partition = dict col, free = rows) so the bias
    is per-partition and fuses into the ACT relu that drains PSUM.
  * Distributed top-k: each core extracts, for every (dict col, 128-row
    sub-chunk), the top-8 values + indices (DVE max8 / max_index).  With
    k/(#sub-chunks) ~ Poisson(0.5) kept elements per sub-chunk, top-8
    covers every globally-kept element a.s.
  * Global merge on host: tau = k-th largest of the candidate union (equal
    to the global k-th largest), then scatter the >= tau candidates into
    the zero output (count-exact at ties, lowest flat index first, matching
    jax.lax.top_k).
"""

import numpy as np

P = 128
N_TOTAL = 4096
K_DIM = 768
DICT = 16384
N_CORES = 8
DICT_SH = DICT // N_CORES     # 2048
KCH = K_DIM // P              # 6
R_BLK = 512
R_BLOCKS = N_TOTAL // R_BLK   # 8
D_TILES = DICT_SH // P        # 16
SPLIT_BITS = 12               # low mantissa bits dropped in the hi part

_cache = {}


def _build_sparse(sub):
    import concourse.mybir as mybir
    import concourse.tile as tile
    from concourse import bacc

    f32 = mybir.dt.float32
    f32r = mybir.dt.float32r
    u32 = mybir.dt.uint32
    NSUB = R_BLK // sub
    CW = 8 * NSUB

    nc = bacc.Bacc("TRN2", target_bir_lowering=False, debug=False,
                   num_devices=N_CORES)
    xT = nc.dram_tensor("xT", [K_DIM, N_TOTAL], f32, kind="ExternalInput")
    wh = nc.dram_tensor("wh", [K_DIM, DICT_SH], f32r, kind="ExternalInput")
    wl = nc.dram_tensor("wl", [K_DIM, DICT_SH], f32r, kind="ExternalInput")
    b = nc.dram_tensor("b", [P, D_TILES], f32, kind="ExternalInput")
    cval = nc.dram_tensor("cval", [DICT_SH, R_BLOCKS * CW], f32,
                          kind="ExternalOutput")
    cidx = nc.dram_tensor("cidx", [DICT_SH, R_BLOCKS * CW], u32,
                          kind="ExternalOutput")

    with tile.TileContext(nc) as tc:
        with (
            tc.tile_pool(name="resident", bufs=1) as rpool,
            tc.tile_pool(name="xraw", bufs=6) as xrpool,
            tc.tile_pool(name="xstream", bufs=2) as xpool,
            tc.tile_pool(name="work", bufs=6) as wpool,
            tc.tile_pool(name="cand", bufs=3) as cpool,
            tc.tile_pool(name="psum", bufs=8, space="PSUM") as psum_pool,
        ):
            wh_sb = rpool.tile([P, KCH, DICT_SH], f32r)
            wl_sb = rpool.tile([P, KCH, DICT_SH], f32r)
            b_sb = rpool.tile([P, D_TILES], f32)
            nc.sync.dma_start(b_sb[:], b.ap())

            xT_r = xT.ap().rearrange("(c p) (rb rr) -> p rb c rr", p=P, rr=R_BLK)
            wh_r = wh.ap().rearrange("(c p) n -> p c n", p=P)
            wl_r = wl.ap().rearrange("(c p) n -> p c n", p=P)

            def load_split_x(r):
                """DMA one f32 r-block of x and split hi/lo on DVE.

                hi = f32r-rounded x (the copy's output rounding), lo = the
                residual, itself f32r-rounded on output; hi + lo carries
                ~22 mantissa bits into the 3-term matmul."""
                xh_t = xpool.tile([P, KCH, R_BLK], f32r, tag="xh")
                xl_t = xpool.tile([P, KCH, R_BLK], f32r, tag="xl")
                for k in range(KCH):
                    xf = xrpool.tile([P, R_BLK], f32, tag="xf")
                    nc.sync.dma_start(xf[:], xT_r[:, r, k])
                    nc.vector.tensor_copy(xh_t[:, k], xf[:])
                    nc.vector.tensor_sub(xl_t[:, k], xf[:],
                                         xh_t[:, k].bitcast(f32))
                return xh_t, xl_t

            # first x block before W so the PE critical path is short; W in
            # d-chunks so early d-tiles can start before the full load lands
            xh_t0, xl_t0 = load_split_x(0)
            # ramped W chunking: fine at the start, coarse after
            edges = [0, 128, 256, 512, 1024, 1536, 2048]
            for q0, q1 in zip(edges[:-1], edges[1:]):
                qs = slice(q0, q1)
                for k in range(KCH):
                    nc.sync.dma_start(wh_sb[:, k, qs], wh_r[:, k, qs])
                    nc.sync.dma_start(wl_sb[:, k, qs], wl_r[:, k, qs])

            for r in range(R_BLOCKS):
                if r == 0:
                    xh_t, xl_t = xh_t0, xl_t0
                else:
                    xh_t, xl_t = load_split_x(r)
                for d in range(D_TILES):
                    ps = psum_pool.tile([P, R_BLK], mybir.dt.float32)
                    dsl = slice(d * P, (d + 1) * P)
                    # hi*hi terms first: the first tiles' matmuls only gate on
                    # the xh copy, so the xl residual computes in their shadow
                    terms = ([(k, wh_sb, xh_t) for k in range(KCH)]
                             + [(k, wh_sb, xl_t) for k in range(KCH)]
                             + [(k, wl_sb, xh_t) for k in range(KCH)])
                    for i, (k, wt, xt) in enumerate(terms):
                        nc.tensor.matmul(
                            ps[:], wt[:, k, dsl], xt[:, k, :],
                            start=(i == 0), stop=(i == len(terms) - 1))
                    sb = wpool.tile([P, R_BLK], f32, tag="sb")
                    nc.scalar.activation(sb[:], ps[:],
                                         mybir.ActivationFunctionType.Relu,
                                         bias=b_sb[:, d:d + 1], scale=1.0)
                    cv = cpool.tile([P, CW], f32, tag="cv")
                    ci = cpool.tile([P, CW], u32, tag="ci")
                    for s in range(NSUB):
                        sl = slice(s * sub, (s + 1) * sub)
                        cs = slice(s * 8, (s + 1) * 8)
                        nc.vector.max(cv[:, cs], sb[:, sl])
                        nc.vector.max_index(ci[:, cs], cv[:, cs], sb[:, sl])
                    nc.sync.dma_start(cval.ap()[dsl, r * CW:(r + 1) * CW], cv[:])
                    nc.sync.dma_start(cidx.ap()[dsl, r * CW:(r + 1) * CW], ci[:])
    nc.compile()
    return nc


def _get_kernel(sub):
    if sub not in _cache:
        _cache[sub] = _build_sparse(sub)
    return _cache[sub]


def _split(a, bits=SPLIT_BITS):
    a = np.ascontiguousarray(a, np.float32)
    hi = (a.view(np.uint32)
          & np.uint32((0xFFFFFFFF << bits) & 0xFFFFFFFF)).view(np.float32)
    return hi, (a - hi).astype(np.float32)


def kernel(x, W_enc, b_enc, top_k):
    from concourse.bass_utils import run_bass_kernel_spmd

    x = np.ascontiguousarray(np.asarray(x), np.float32)
    W_enc = np.ascontiguousarray(np.asarray(W_enc), np.float32)
    b_enc = np.ascontiguousarray(np.asarray(b_enc), np.float32).ravel()
    top_k = int(np.asarray(top_k))
    k_tot = top_k * x.shape[0]
    out = np.zeros((N_TOTAL, DICT), np.float32)
    if k_tot <= 0:
        return out

    # sub-chunk size: expected kept per sub-chunk is top_k * sub / DICT;
    # top-8 per sub-chunk needs that (plus tail) well under 8.
    if top_k <= 96:
        sub = 128
    elif top_k <= 192:
        sub = 64
    else:
        sub = 32
    NSUB = R_BLK // sub
    CW = 8 * NSUB

    nc = _get_kernel(sub)

    xT = np.ascontiguousarray(x.T)
    ins = []
    for c in range(N_CORES):
        sl = slice(c * DICT_SH, (c + 1) * DICT_SH)
        wsh = np.ascontiguousarray(W_enc[:, sl])
        whi, wlo = _split(wsh)
        bsh = np.ascontiguousarray(b_enc[sl]).reshape(D_TILES, P).T.copy()
        ins.append({"xT": xT, "wh": whi, "wl": wlo, "b": bsh})

    try:
        res = run_bass_kernel_spmd(nc, ins, core_ids=list(range(N_CORES)))
    except Exception:
        # transient device errors (e.g. NRT_EXEC_UNIT_UNRECOVERABLE) recover
        # on re-execution; one retry
        res = run_bass_kernel_spmd(nc, ins, core_ids=list(range(N_CORES)))

    # ---- global merge (host): exact tau + count-exact scatter ----
    vals = np.stack([res.results[c]["cval"] for c in range(N_CORES)])
    idxs = np.stack([res.results[c]["cidx"] for c in range(N_CORES)])
    flat = vals.ravel()
    k_eff = min(k_tot, flat.size)
    tau = np.partition(flat, -k_eff)[-k_eff]

    sel = flat >= tau
    fidx = np.flatnonzero(sel)
    v = flat[fidx]
    ii = idxs.ravel()[fidx].astype(np.int64)
    c_, rem = np.divmod(fidx, DICT_SH * R_BLOCKS * CW)
    dcol, rem = np.divmod(rem, R_BLOCKS * CW)
    rb, rem = np.divmod(rem, CW)
    s, _ = np.divmod(rem, 8)
    row = rb * R_BLK + s * sub + ii
    col = c_ * DICT_SH + dcol

    if tau > 0:
        n_gt = int((v > tau).sum())
        need = k_tot - n_gt
        at_tau = np.flatnonzero(v == tau)
        if at_tau.size > need:
            # count-exact tie-break: keep lowest flat index, like lax.top_k
            order = np.argsort(row[at_tau] * DICT + col[at_tau], kind="stable")
            drop = at_tau[order[need:]]
            keep = np.ones(v.size, bool)
            keep[drop] = False
            v, row, col = v[keep], row[keep], col[keep]
    else:
        # k exceeds the positive count: only positive values are visible
        # (setting zeros at zero positions is a no-op)
        keep = v > 0
        v, row, col = v[keep], row[keep], col[keep]

    out[row, col] = v
    return out



# revision 2
# speedup vs baseline: 1.9304x; 1.9304x over previous
"""CrossLayerTranscoder with global batch-wise top-k masking on 8 TRN2 cores.

Reference computation:
    pre = relu(x @ W_enc + b_enc)            [4096, 16384]
    keep the global top-(top_k * 4096) entries, zero the rest.

Device algorithm (single pass, dict-sharded over 8 cores):
  * GEMM in single-term f32r precision (PE rounds the f32 inputs to its
    ~11-bit multiplier internally): 1 cycle/row instead of fp32's 4 or
    split-f32r's 3.  Differential error on pre_acts is ~2e-4 absolute.
  * Transposed orientation (partition = dict col, free = rows) so the bias
    is per-partition and fuses into the ACT relu that drains PSUM.
  * Distributed top-k: each core extracts, for every (dict col, 128-row
    sub-chunk), the top-8 values + indices (DVE max8 / max_index).  With
    k/(#sub-chunks) ~ Poisson(0.5) kept elements per sub-chunk, top-8
    covers every globally-kept element a.s.
  * Global merge on host: tau_hat = k-th largest candidate.  Candidates
    confidently above tau_hat keep their device (approximate) value —
    value noise of ~2e-4 on ~2.3-magnitude entries is far inside the
    error budget.  Candidates inside a +-DELTA band around tau_hat get
    their dot product recomputed exactly (fp64) on host, and the exact
    values decide the final boundary of the top-k set — so the selected
    SET matches the reference's, which is what the error metric is
    actually sensitive to (a swapped element costs ~2*tau^2 in norm^2).
"""

import numpy as np

P = 128
N_TOTAL = 4096
K_DIM = 768
DICT = 16384
N_CORES = 8
DICT_SH = DICT // N_CORES     # 2048
KCH = K_DIM // P              # 6
R_BLK = 512
R_BLOCKS = N_TOTAL // R_BLK   # 8
D_TILES = DICT_SH // P        # 16
DELTA = 3e-3                  # band half-width around tau_hat (~18 sigma)

_cache = {}


def _build_sparse(sub):
    import concourse.mybir as mybir
    import concourse.tile as tile
    from concourse import bacc

    f32 = mybir.dt.float32
    f32r = mybir.dt.float32r
    u32 = mybir.dt.uint32
    NSUB = R_BLK // sub
    CW = 8 * NSUB

    nc = bacc.Bacc("TRN2", target_bir_lowering=False, debug=False,
                   num_devices=N_CORES)
    xT = nc.dram_tensor("xT", [K_DIM, N_TOTAL], f32r, kind="ExternalInput")
    wh = nc.dram_tensor("wh", [K_DIM, DICT_SH], f32r, kind="ExternalInput")
    b = nc.dram_tensor("b", [P, D_TILES], f32, kind="ExternalInput")
    cval = nc.dram_tensor("cval", [DICT_SH, R_BLOCKS * CW], f32,
                          kind="ExternalOutput")
    cidx = nc.dram_tensor("cidx", [DICT_SH, R_BLOCKS * CW], u32,
                          kind="ExternalOutput")

    with tile.TileContext(nc) as tc:
        with (
            tc.tile_pool(name="resident", bufs=1) as rpool,
            tc.tile_pool(name="xstream", bufs=2) as xpool,
            tc.tile_pool(name="work", bufs=6) as wpool,
            tc.tile_pool(name="cand", bufs=3) as cpool,
            tc.tile_pool(name="psum", bufs=8, space="PSUM") as psum_pool,
        ):
            wh_sb = rpool.tile([P, KCH, DICT_SH], f32r)
            b_sb = rpool.tile([P, D_TILES], f32)
            nc.sync.dma_start(b_sb[:], b.ap())

            xT_r = xT.ap().rearrange("(c p) (rb rr) -> p rb c rr", p=P, rr=R_BLK)
            wh_r = wh.ap().rearrange("(c p) n -> p c n", p=P)

            def load_x(r):
                xh_t = xpool.tile([P, KCH, R_BLK], f32r, tag="xh")
                for k in range(KCH):
                    nc.sync.dma_start(xh_t[:, k], xT_r[:, r, k])
                return xh_t

            # first x block before W so the PE critical path is short; W in
            # d-chunks so early d-tiles can start before the full load lands
            xh_t0 = load_x(0)
            # ramped W chunking: fine at the start, coarse after
            edges = [0, 128, 256, 512, 1024, 1536, 2048]
            for q0, q1 in zip(edges[:-1], edges[1:]):
                qs = slice(q0, q1)
                for k in range(KCH):
                    nc.sync.dma_start(wh_sb[:, k, qs], wh_r[:, k, qs])

            for r in range(R_BLOCKS):
                xh_t = xh_t0 if r == 0 else load_x(r)
                for d in range(D_TILES):
                    ps = psum_pool.tile([P, R_BLK], mybir.dt.float32)
                    dsl = slice(d * P, (d + 1) * P)
                    for k in range(KCH):
                        nc.tensor.matmul(
                            ps[:], wh_sb[:, k, dsl], xh_t[:, k, :],
                            start=(k == 0), stop=(k == KCH - 1))
                    sb = wpool.tile([P, R_BLK], f32, tag="sb")
                    nc.scalar.activation(sb[:], ps[:],
                                         mybir.ActivationFunctionType.Relu,
                                         bias=b_sb[:, d:d + 1], scale=1.0)
                    cv = cpool.tile([P, CW], f32, tag="cv")
                    ci = cpool.tile([P, CW], u32, tag="ci")
                    for s in range(NSUB):
                        sl = slice(s * sub, (s + 1) * sub)
                        cs = slice(s * 8, (s + 1) * 8)
                        nc.vector.max(cv[:, cs], sb[:, sl])
                        nc.vector.max_index(ci[:, cs], cv[:, cs], sb[:, sl])
                    nc.sync.dma_start(cval.ap()[dsl, r * CW:(r + 1) * CW], cv[:])
                    nc.sync.dma_start(cidx.ap()[dsl, r * CW:(r + 1) * CW], ci[:])
    nc.compile()
    return nc


def _get_kernel(sub):
    if sub not in _cache:
        _cache[sub] = _build_sparse(sub)
    return _cache[sub]


def kernel(x, W_enc, b_enc, top_k):
    from concourse.bass_utils import run_bass_kernel_spmd

    x = np.ascontiguousarray(np.asarray(x), np.float32)
    W_enc = np.ascontiguousarray(np.asarray(W_enc), np.float32)
    b_enc = np.ascontiguousarray(np.asarray(b_enc), np.float32).ravel()
    top_k = int(np.asarray(top_k))
    k_tot = top_k * x.shape[0]
    out = np.zeros((N_TOTAL, DICT), np.float32)
    if k_tot <= 0:
        return out

    # sub-chunk size: expected kept per sub-chunk is top_k * sub / DICT;
    # top-8 per sub-chunk needs that (plus tail) well under 8.
    if top_k <= 96:
        sub = 128
    elif top_k <= 192:
        sub = 64
    else:
        sub = 32
    NSUB = R_BLK // sub
    CW = 8 * NSUB

    nc = _get_kernel(sub)

    xT = np.ascontiguousarray(x.T)
    ins = []
    for c in range(N_CORES):
        sl = slice(c * DICT_SH, (c + 1) * DICT_SH)
        wsh = np.ascontiguousarray(W_enc[:, sl])
        bsh = np.ascontiguousarray(b_enc[sl]).reshape(D_TILES, P).T.copy()
        ins.append({"xT": xT, "wh": wsh, "b": bsh})

    try:
        res = run_bass_kernel_spmd(nc, ins, core_ids=list(range(N_CORES)))
    except Exception:
        # transient device errors (e.g. NRT_EXEC_UNIT_UNRECOVERABLE) recover
        # on re-execution; one retry
        res = run_bass_kernel_spmd(nc, ins, core_ids=list(range(N_CORES)))

    # ---- global merge (host) ----
    vals = np.stack([res.results[c]["cval"] for c in range(N_CORES)])
    idxs = np.stack([res.results[c]["cidx"] for c in range(N_CORES)])
    flat = vals.ravel()
    k_eff = min(k_tot, flat.size)
    tau_hat = np.partition(flat, -k_eff)[-k_eff]

    # decode every candidate that could possibly be in the top-k set
    sel = flat >= tau_hat - DELTA
    fidx = np.flatnonzero(sel)
    v = flat[fidx]
    ii = idxs.ravel()[fidx].astype(np.int64)
    c_, rem = np.divmod(fidx, DICT_SH * R_BLOCKS * CW)
    dcol, rem = np.divmod(rem, R_BLOCKS * CW)
    rb, rem = np.divmod(rem, CW)
    s, _ = np.divmod(rem, 8)
    row = rb * R_BLK + s * sub + ii
    col = c_ * DICT_SH + dcol

    if tau_hat > DELTA:
        conf = v > tau_hat + DELTA
        n_conf = int(conf.sum())
        out[row[conf], col[conf]] = v[conf]

        band = ~conf
        br, bc = row[band], col[band]
        exact = np.einsum(
            "ij,ij->i",
            x[br].astype(np.float64),
            W_enc[:, bc].T.astype(np.float64),
        ) + b_enc[bc].astype(np.float64)
        exact = np.maximum(exact, 0.0)
        need = k_tot - n_conf
        if need > 0:
            # exact values decide the boundary; ties -> lowest flat index,
            # matching jax.lax.top_k
            order = np.lexsort((br * DICT + bc, -exact))
            kept = order[:need]
            out[br[kept], bc[kept]] = exact[kept].astype(np.float32)
    else:
        # k exceeds the positive count: only positive values are visible
        # (setting zeros at zero positions is a no-op)
        keep = v > 0
        out[row[keep], col[keep]] = v[keep]
    return out


# revision 4
# speedup vs baseline: 2.8866x; 1.4953x over previous
"""CrossLayerTranscoder with global batch-wise top-k masking on 8 TRN2 cores.

Reference computation:
    pre = relu(x @ W_enc + b_enc)            [4096, 16384]
    keep the global top-(top_k * 4096) entries, zero the rest.

Device algorithm (single pass, dict-sharded over 8 cores):
  * GEMM in single-term bf16: 1 cycle/row on the PE (same rate as f32r,
    half the DMA/LDWEIGHTS cost).  Differential noise on pre_acts is
    ~1.6e-3 rms; everything accuracy-critical is repaired on the host.
  * No bias / relu on device: bias is constant within a dict column, so
    the per-(col, row-block) top-8 is unchanged by it; the host adds
    b_enc[col] to the candidate values.  relu never changes the top-k
    set when tau > 0.
  * Transposed orientation (partition = dict col, free = rows).  The DVE
    extracts, for every (dict col, 512-row block) PSUM tile, the top-8
    values + indices (MAX8 / FIND_INDEX8 straight from PSUM, one pass
    each).
  * Global merge on host:
      - tau_hat = k-th largest biased candidate.
      - 'saturated' (col, block) chunks - those whose 8th candidate is
        still >= tau_hat - DELTA - might hide further top-k members
        beyond the 8 extracted; the host recomputes those chunks' dot
        products exactly (~200 chunks).
      - candidates within +-DELTA of tau_hat get exact recomputation
        too; the exact values decide the top-k boundary, so the
        selected SET matches the reference's (a swapped element costs
        ~2*tau^2 in norm^2, which is what the metric is sensitive to).
      - confident candidates (> tau_hat + DELTA) keep the device value;
        its ~1.6e-3 noise is far inside the error budget.
"""

import numpy as np

P = 128
N_TOTAL = 4096
K_DIM = 768
DICT = 16384
N_CORES = 8
DICT_SH = DICT // N_CORES     # 2048
KCH = K_DIM // P              # 6
R_BLK = 512
R_BLOCKS = N_TOTAL // R_BLK   # 8
D_TILES = DICT_SH // P        # 16
CW = 8                        # top-8 per (col, 512-row block)
DELTA = 1.2e-2                # band half-width around tau_hat (~7.5 sigma)

_cache = {}


def _build_sparse():
    import concourse.mybir as mybir
    import concourse.tile as tile
    from concourse import bacc

    f32 = mybir.dt.float32
    bf16 = mybir.dt.bfloat16
    u32 = mybir.dt.uint32

    nc = bacc.Bacc("TRN2", target_bir_lowering=False, debug=False,
                   num_devices=N_CORES)
    xT = nc.dram_tensor("xT", [K_DIM, N_TOTAL], bf16, kind="ExternalInput")
    wh = nc.dram_tensor("wh", [K_DIM, DICT_SH], bf16, kind="ExternalInput")
    cval = nc.dram_tensor("cval", [R_BLOCKS * P, D_TILES * CW], f32,
                          kind="ExternalOutput")
    cidx = nc.dram_tensor("cidx", [R_BLOCKS * P, D_TILES * CW], u32,
                          kind="ExternalOutput")

    with tile.TileContext(nc) as tc:
        with (
            tc.tile_pool(name="resident", bufs=1) as rpool,
            tc.tile_pool(name="xstream", bufs=2) as xpool,
            tc.tile_pool(name="cand", bufs=2) as cpool,
            tc.tile_pool(name="psum", bufs=8, space="PSUM") as psum_pool,
        ):
            wh_sb = rpool.tile([P, KCH, DICT_SH], bf16)

            xT_r = xT.ap().rearrange("(c p) (rb rr) -> p rb c rr", p=P, rr=R_BLK)
            wh_r = wh.ap().rearrange("(c p) n -> p c n", p=P)
            cval_r = cval.ap().rearrange("(rb p) w -> p rb w", p=P)
            cidx_r = cidx.ap().rearrange("(rb p) w -> p rb w", p=P)

            def load_x(r):
                xh_t = xpool.tile([P, KCH, R_BLK], bf16, tag="xh")
                for k in range(KCH):
                    nc.sync.dma_start(xh_t[:, k], xT_r[:, r, k])
                return xh_t

            # first x block before W so the PE critical path is short; W in
            # d-chunks so early d-tiles can start before the full load lands
            xh_t0 = load_x(0)
            # ramped W chunking: fine at the start, coarse after
            edges = [0, 128, 256, 512, 1024, 1536, 2048]
            for q0, q1 in zip(edges[:-1], edges[1:]):
                qs = slice(q0, q1)
                for k in range(KCH):
                    nc.sync.dma_start(wh_sb[:, k, qs], wh_r[:, k, qs])

            for r in range(R_BLOCKS):
                xh_t = xh_t0 if r == 0 else load_x(r)
                cvb = cpool.tile([P, D_TILES, CW], f32, tag="cv")
                cib = cpool.tile([P, D_TILES, CW], u32, tag="ci")
                for d in range(D_TILES):
                    ps = psum_pool.tile([P, R_BLK], mybir.dt.float32)
                    dsl = slice(d * P, (d + 1) * P)
                    for k in range(KCH):
                        nc.tensor.matmul(
                            ps[:], wh_sb[:, k, dsl], xh_t[:, k, :],
                            start=(k == 0), stop=(k == KCH - 1))
                    nc.vector.max(cvb[:, d], ps[:])
                    nc.vector.max_index(cib[:, d], cvb[:, d], ps[:])
                nc.sync.dma_start(cval_r[:, r], cvb[:])
                nc.sync.dma_start(cidx_r[:, r], cib[:])
    nc.compile()
    return nc


def _get_kernel():
    if "k" not in _cache:
        _cache["k"] = _build_sparse()
    return _cache["k"]


def kernel(x, W_enc, b_enc, top_k):
    import ml_dtypes
    from concourse.bass_utils import run_bass_kernel_spmd

    x = np.ascontiguousarray(np.asarray(x), np.float32)
    W_enc = np.ascontiguousarray(np.asarray(W_enc), np.float32)
    b_enc = np.ascontiguousarray(np.asarray(b_enc), np.float32).ravel()
    top_k = int(np.asarray(top_k))
    k_tot = top_k * x.shape[0]
    out = np.zeros((N_TOTAL, DICT), np.float32)
    if k_tot <= 0:
        return out

    nc = _get_kernel()

    xT = np.ascontiguousarray(x.T.astype(ml_dtypes.bfloat16))
    W16 = W_enc.astype(ml_dtypes.bfloat16)
    ins = []
    for c in range(N_CORES):
        sl = slice(c * DICT_SH, (c + 1) * DICT_SH)
        ins.append({"xT": xT, "wh": np.ascontiguousarray(W16[:, sl])})

    try:
        res = run_bass_kernel_spmd(nc, ins, core_ids=list(range(N_CORES)))
    except Exception:
        # transient device errors (e.g. NRT_EXEC_UNIT_UNRECOVERABLE) recover
        # on re-execution; one retry
        res = run_bass_kernel_spmd(nc, ins, core_ids=list(range(N_CORES)))

    # ---- global merge (host) ----
    # flat layout: [core, rb, p, d, slot];  col = c*2048 + d*128 + p,
    # row = rb*512 + idx
    vals = np.stack([res.results[c]["cval"] for c in range(N_CORES)])
    idxs = np.stack([res.results[c]["cidx"] for c in range(N_CORES)])
    flat = vals.ravel()
    ii = idxs.ravel().astype(np.int64)

    n_flat = flat.size
    f = np.arange(n_flat, dtype=np.int64)
    c_, rem = np.divmod(f, R_BLOCKS * P * D_TILES * CW)
    rb, rem = np.divmod(rem, P * D_TILES * CW)
    p, rem = np.divmod(rem, D_TILES * CW)
    d, slot = np.divmod(rem, CW)
    col = (c_ * DICT_SH + d * P + p).astype(np.int64)
    row = rb * R_BLK + ii

    vb = flat + b_enc[col]                    # biased candidate values

    k_eff = min(k_tot, n_flat)
    tau_hat = float(np.partition(vb, -k_eff)[-k_eff])

    if tau_hat <= DELTA:
        # degenerate regime (k >= positive count): values near zero,
        # approximate selection is fine
        keep = vb > 0
        order = np.argsort(-vb[keep])[:k_tot]
        kr, kc = row[keep][order], col[keep][order]
        out[kr, kc] = np.maximum(vb[keep][order], 0)
        return out

    # chunk = (core, dcol, rb) <-> flat // CW; slot 7 is the chunk's 8th
    # (smallest extracted) value: if it is still near/above the threshold
    # the chunk may hide more top-k members beyond the extracted 8.
    v8 = vb[slot == 7]
    sat_chunk = np.flatnonzero(v8 >= tau_hat - DELTA)   # chunk ids
    chunk_id = f // CW
    in_sat = np.isin(chunk_id, sat_chunk)

    conf = (vb > tau_hat + DELTA) & ~in_sat
    band = (vb >= tau_hat - DELTA) & (vb <= tau_hat + DELTA) & ~in_sat

    # exact recompute pool: all rows of saturated chunks + band candidates
    er_list = [row[band]]
    ec_list = [col[band]]
    if sat_chunk.size:
        # chunk id -> (core, dcol-part, rb) -> col, row range
        sc_, srem = np.divmod(sat_chunk, R_BLOCKS * P * D_TILES)
        srb, srem = np.divmod(srem, P * D_TILES)
        sp, sd = np.divmod(srem, D_TILES)
        scol = sc_ * DICT_SH + sd * P + sp
        er_list.append(
            (srb[:, None] * R_BLK + np.arange(R_BLK)[None, :]).ravel())
        ec_list.append(np.repeat(scol, R_BLK))
    er = np.concatenate(er_list)
    ec = np.concatenate(ec_list)
    # dedupe exact positions (tied FIND_INDEX8 needles can repeat)
    epos = er * DICT + ec
    epos, uq = np.unique(epos, return_index=True)
    er, ec = er[uq], ec[uq]

    ev = np.empty(er.size, np.float32)
    CH = 65536
    for i in range(0, er.size, CH):
        s = slice(i, i + CH)
        ev[s] = np.einsum("ij,ij->i", x[er[s]], W_enc[:, ec[s]].T,
                          optimize=True) + b_enc[ec[s]]

    # confident candidates are all truly in the top-k (their true value is
    # > tau_hat + DELTA - noise > tau); duplicated positions carry
    # identical values, so plain assignment is safe
    out[row[conf], col[conf]] = vb[conf]
    n_conf = np.unique(row[conf] * DICT + col[conf]).size

    need = k_tot - n_conf
    if need > 0:
        # exact values decide the boundary; ties -> lowest flat index,
        # matching jax.lax.top_k
        order = np.lexsort((epos, -ev.astype(np.float64)))
        kept = order[:need]
        out[er[kept], ec[kept]] = np.maximum(ev[kept], 0)
    return out
